# revision 8
# baseline (speedup 1.0000x reference)
"""Trainium2 fused Bass kernel for nn_ArrowTransformer (B=2,S=1024,D=1024,H=16,L=6,V=256).

Single fused NEFF across 8 NeuronCores (one SPMD graph, one invocation):
  - Attention is head-split: core c owns heads {2c, 2c+1} for all 2048
    positions. QKV projections use the full h^T (AllGathered per layer) with
    per-core weight column slices.
  - Music-Transformer relative attention (Srel) is realized with an augmented
    q/E matmul: q_aug = [q; 1], E_aug rows >= S are (0, -1e10), so the skewed
    re-read of the Q@E^T scratch (strided DRAM round trip) yields -1e10 in
    exactly the causally-masked (j > p) entries -- no separate masks needed.
    Pad-key masking rides the exp() activation's per-partition bias (-30).
  - Softmax denominator via augmented V (ones column); the division is folded
    in as a PE ones-matmul broadcast of 1/den.
  - After attention, an AllToAll reshards attnT [2 heads x 2048] -> full
    [1024 x 256 own positions]; Wo + LN1 + FFN + LN2 run position-locally;
    an AllGather of h^T feeds the next layer. All matmuls bf16 with fp32 PSUM.
"""

import math
import os
import time

import numpy as np

B, S, D, H, L, V = 2, 1024, 1024, 16, 6, 256
HD = D // H  # 64
NC = 8
PP = (B * S) // NC  # 256 own positions per core
QB = S // 128  # 8 query blocks per batch
NEG_E = -1.0e10  # srel value for masked (j>p) entries; *1/8 after scale
PAD_BIAS = -30.0  # pad-key mask as exp bias (softmax-exact for all-pad rows)

_G = {}
EXEC_NS = [0]
LAST = {}


def _pos_encoding():
    i = np.arange(D, dtype=np.float64)
    par = np.mod(i, 2.0)
    rate = np.exp(-math.log(10000.0) * i / D) * np.exp(math.log(10000.0) * par / D)
    pos = np.arange(S, dtype=np.float64)
    return np.sin(pos[:, None] * rate[None, :] + 0.5 * math.pi * par[None, :]).astype(
        np.float32
    )


def _legalize_sync_waits(nc):
    """walrus on this image allows 1 sync-wait per instruction (2 on
    EventSemaphore); split longer on_wait lists into preceding event-sem
    instructions on the same engine."""
    import concourse.mybir as mybir

    cnt = 0
    for fn in nc.m.functions:
        for blk in fn.blocks:
            insts = list(blk.instructions)
            out = []
            changed = False
            for inst in insts:
                si = inst.sync_info
                waits = list(si.on_wait) if (si and si.on_wait) else []
                allowed = 2 if isinstance(inst, mybir.InstEventSemaphore) else 1
                if len(waits) > allowed:
                    changed = True
                    extra, keep = waits[:-allowed], waits[-allowed:]
                    for i in range(0, len(extra), 2):
                        cnt += 1
                        out.append(
                            mybir.InstEventSemaphore(
                                name=f"legw_{cnt}_{inst.name}",
                                engine=inst.engine,
                                sync_info=mybir.SyncInfo(
                                    on_wait=extra[i : i + 2], on_update=[]
                                ),
                                ins=[],
                                outs=[],
                            )
                        )
                    inst.sync_info = mybir.SyncInfo(
                        on_wait=keep,
                        on_update=list(si.on_update) if si.on_update else [],
                    )
                out.append(inst)
            if changed:
                blk.instructions = out
    return cnt


# scratch geometry: per (head, batch, qi): width W(qi) = 255 + 128*qi, rows 128
def _qe_widths():
    return [255 + 128 * qi for qi in range(QB)]


def _build(nl):
    import concourse.bass as bass
    import concourse.mybir as mybir
    import concourse.tile as tile

    f32 = mybir.dt.float32
    bf16 = mybir.dt.bfloat16
    nc = bass.Bass(num_devices=NC)

    hT0_d = nc.dram_tensor("hT0", [D, B * S], bf16, kind="ExternalInput")
    wq_d = nc.dram_tensor("wq", [nl, D, 2 * HD], bf16, kind="ExternalInput")
    wk_d = nc.dram_tensor("wk", [nl, D, 2 * HD], bf16, kind="ExternalInput")
    wv_d = nc.dram_tensor("wv", [nl, D, 2 * HD], bf16, kind="ExternalInput")
    bqkv_d = nc.dram_tensor("bqkv", [nl, 128, 3], f32, kind="ExternalInput")
    wo_d = nc.dram_tensor("wo", [nl, D, D], bf16, kind="ExternalInput")
    w1_d = nc.dram_tensor("w1", [nl, D, D // 2], bf16, kind="ExternalInput")
    w2_d = nc.dram_tensor("w2", [nl, D // 2, D], bf16, kind="ExternalInput")
    eT_d = nc.dram_tensor("eT", [nl, HD + 1, 1151], bf16, kind="ExternalInput")
    vecs_d = nc.dram_tensor("vecs", [nl, 128, 6656], bf16, kind="ExternalInput")
    pad_d = nc.dram_tensor("padneg", [128, 2 * QB], f32, kind="ExternalInput")
    id_d = nc.dram_tensor("ident", [128, 128], bf16, kind="ExternalInput")
    wf_d = nc.dram_tensor("wf", [D, V], bf16, kind="ExternalInput")
    bf_d = nc.dram_tensor("bfv", [128, V], f32, kind="ExternalInput")
    out_d = nc.dram_tensor("logits", [PP, V], f32, kind="ExternalOutput")

    Ws = _qe_widths()
    qe_base = []  # flat elem offset per (hh, b, qi)
    off = 0
    for hh in range(2):
        for b in range(B):
            for qi in range(QB):
                qe_base.append(off)
                off += 128 * Ws[qi]
    qe_d = nc.dram_tensor("qe_scratch", [off], bf16, kind="Internal")
    a2a_in = nc.dram_tensor("a2a_in", [NC, 128, PP], bf16, kind="Internal")
    a2a_out = nc.dram_tensor("a2a_out", [NC, 128, PP], bf16, kind="Internal")
    ag_in = nc.dram_tensor("ag_in", [D, PP], bf16, kind="Internal")
    ag_out = nc.dram_tensor(
        "ag_out", [NC, D, PP], bf16, kind="Internal", addr_space="Shared"
    )

    def qe_ap(hh, b, qi, extra_off, steps):
        base = qe_d[:]
        return bass.AP(base.tensor, base.offset + qe_base[(hh * B + b) * QB + qi] + extra_off, steps)

    with tile.TileContext(nc) as tc:
        with (
            tc.tile_pool(name="hpool", bufs=1) as hpool,
            tc.tile_pool(name="wpool", bufs=2) as wpool,
            tc.tile_pool(name="w2pool", bufs=1) as w2pool,
            tc.tile_pool(name="apool", bufs=1) as apool,
            tc.tile_pool(name="qepool", bufs=2) as qepool,
            tc.tile_pool(name="tpool", bufs=4) as tpool,
            tc.tile_pool(name="lnpool", bufs=1) as lnpool,
            tc.tile_pool(name="cpool", bufs=1) as cpool,
            tc.tile_pool(name="ps_mm", bufs=2, space="PSUM") as ps_mm,
            tc.tile_pool(name="ps_qk", bufs=2, space="PSUM") as ps_qk,
            tc.tile_pool(name="ps_tr", bufs=2, space="PSUM") as ps_tr,
            tc.tile_pool(name="ps_at", bufs=2, space="PSUM") as ps_at,
        ):
            # constants
            id_sb = cpool.tile([128, 128], bf16, tag="id")
            nc.sync.dma_start(id_sb[:], id_d[:])
            pad_sb = cpool.tile([128, 2 * QB], f32, tag="pad")
            nc.sync.dma_start(pad_sb[:], pad_d[:])
            ones_sb = cpool.tile([1, 64], bf16, tag="ones")
            nc.vector.memset(ones_sb[:], 1.0)
            bfb_sb = cpool.tile([128, V], f32, tag="bfb")
            nc.sync.dma_start(bfb_sb[:], bf_d[:])

            hT = hpool.tile([128, QB, B * S], bf16, tag="hT")
            for kc in range(QB):
                nc.sync.dma_start(
                    hT[:, kc, :],
                    bass.AP(hT0_d[:].tensor, hT0_d[:].offset + kc * 128 * B * S,
                            [[B * S, 128], [1, B * S]]),
                )

            hT_own = None
            for l in range(nl):
                # ---- per-layer weights ----
                wqkv = wpool.tile([128, QB, 3, 2 * HD], bf16, tag="wqkv")
                for i, wd in enumerate((wq_d, wk_d, wv_d)):
                    a = wd[l]
                    nc.sync.dma_start(
                        wqkv[:, :, i, :],
                        bass.AP(a.tensor, a.offset, [[2 * HD, 128], [128 * 2 * HD, QB], [1, 2 * HD]]),
                    )
                bqkv = wpool.tile([128, 3], f32, tag="bqkv")
                nc.sync.dma_start(bqkv[:], bqkv_d[l])
                wo_sb = w2pool.tile([128, QB, D], bf16, tag="wo")
                a = wo_d[l]
                nc.sync.dma_start(
                    wo_sb[:], bass.AP(a.tensor, a.offset, [[D, 128], [128 * D, QB], [1, D]])
                )
                w1_sb = wpool.tile([128, QB, D // 2], bf16, tag="w1")
                a = w1_d[l]
                nc.sync.dma_start(
                    w1_sb[:], bass.AP(a.tensor, a.offset, [[D // 2, 128], [128 * D // 2, QB], [1, D // 2]])
                )
                w2_sb = wpool.tile([128, 4, D], bf16, tag="w2")
                a = w2_d[l]
                nc.sync.dma_start(
                    w2_sb[:], bass.AP(a.tensor, a.offset, [[D, 128], [128 * D, 4], [1, D]])
                )
                eT_sb = wpool.tile([HD + 1, 1151], bf16, tag="eT")
                nc.sync.dma_start(eT_sb[:], eT_d[l])
                vbc = w2pool.tile([128, 6656], bf16, tag="vbc")
                nc.sync.dma_start(vbc[:], vecs_d[l])
                VOF = dict(bo=0, g1=1024, be1=2048, b2=3072, g2=4096, be2=5120, b1=6144)

                # ---- QKV for my 2 heads, all positions ----
                q_aug = apool.tile([HD + 1, 2, B * S], bf16, tag="q_aug")
                nc.vector.memset(q_aug[HD : HD + 1, :, :], 1.0)
                kT = apool.tile([64, 2, B * S], bf16, tag="kT")
                vT = apool.tile([128, 4, 512], bf16, tag="vT_atf")
                for piece in range(4):
                    psl = slice(piece * 512, (piece + 1) * 512)
                    for i in range(3):
                        ps = ps_mm.tile([128, 512], f32, tag="mm")
                        for kc in range(QB):
                            nc.tensor.matmul(
                                ps[:], wqkv[:, kc, i, :], hT[:, kc, psl],
                                start=(kc == 0), stop=(kc == QB - 1),
                            )
                        if i == 0:
                            for hh in range(2):
                                nc.vector.tensor_tensor(
                                    q_aug[0:HD, hh, psl], ps[hh * HD : (hh + 1) * HD, :],
                                    bqkv[hh * HD : (hh + 1) * HD, 0:1].to_broadcast([HD, 512]),
                                    mybir.AluOpType.add,
                                )
                        elif i == 1:
                            for hh in range(2):
                                nc.vector.tensor_tensor(
                                    kT[:, hh, psl], ps[hh * HD : (hh + 1) * HD, :],
                                    bqkv[hh * HD : (hh + 1) * HD, 1:2].to_broadcast([HD, 512]),
                                    mybir.AluOpType.add,
                                )
                        else:
                            nc.vector.tensor_tensor(
                                vT[:, piece, :], ps[:],
                                bqkv[:, 2:3].to_broadcast([128, 512]),
                                mybir.AluOpType.add,
                            )
                # v_aug [j, 16 jb-chunks, 130]: per head 65 cols (64 + ones)
                v_aug = apool.tile([128, 2 * QB, 130], bf16, tag="v_aug")
                nc.vector.memset(v_aug[:, :, HD : HD + 1], 1.0)
                nc.vector.memset(v_aug[:, :, 129:130], 1.0)
                for t in range(2 * QB):
                    ptr = ps_tr.tile([128, 128], bf16, tag="ptr")
                    nc.tensor.transpose(
                        ptr[:], vT[:, t // 4, (t % 4) * 128 : (t % 4) * 128 + 128], id_sb[:]
                    )
                    nc.vector.tensor_copy(v_aug[:, t, 0:HD], ptr[:, 0:HD])
                    nc.vector.tensor_copy(v_aug[:, t, 65 : 65 + HD], ptr[:, HD:128])

                # ---- attention: QE scratch + tiles ----
                attnT = apool.tile([128, B * S], bf16, tag="attnT")
                for hh in range(2):
                    for b in range(B):
                        for qi in range(QB):
                            W = Ws[qi]
                            t0 = qi * 128
                            e0 = 896 - t0
                            qcols = slice(b * S + t0, b * S + t0 + 128)
                            qsl = q_aug[:, hh, qcols]
                            qe_sb = qepool.tile([128, 1151], bf16, tag="qe_sb")
                            npiece = (W + 511) // 512
                            for p in range(npiece):
                                w0 = p * 512
                                w1 = min(W, w0 + 512)
                                pqe = ps_mm.tile([128, 512], f32, tag="mm")
                                nc.tensor.matmul(
                                    pqe[:, 0 : w1 - w0], qsl,
                                    eT_sb[:, e0 + w0 : e0 + w1],
                                    start=True, stop=True,
                                )
                                nc.vector.tensor_copy(qe_sb[:, w0:w1], pqe[:, 0 : w1 - w0])
                            nc.sync.dma_start(
                                qe_ap(hh, b, qi, 0, [[W, 128], [1, W]]),
                                qe_sb[:, 0:W],
                            )
                            # tiles
                            pat = ps_at.tile([128, 128], f32, tag="pat")
                            for jb in range(qi + 1):
                                pqk = ps_qk.tile([128, 128], f32, tag="qk")
                                nc.tensor.matmul(
                                    pqk[:],
                                    kT[:, hh, b * S + jb * 128 : b * S + jb * 128 + 128],
                                    q_aug[0:HD, hh, qcols],
                                    start=True, stop=True,
                                )
                                srel = tpool.tile([128, 128], bf16, tag="srel")
                                nc.sync.dma_start(
                                    srel[:],
                                    qe_ap(hh, b, qi, 127 + 128 * jb, [[1, 128], [W - 1, 128]]),
                                )
                                x_sb = tpool.tile([128, 128], f32, tag="x")
                                nc.vector.tensor_tensor(
                                    x_sb[:], pqk[:], srel[:], mybir.AluOpType.add
                                )
                                ex = tpool.tile([128, 128], bf16, tag="ex")
                                nc.scalar.activation(
                                    ex[:], x_sb[:], mybir.ActivationFunctionType.Exp,
                                    scale=1.0 / math.sqrt(HD),
                                    bias=pad_sb[:, b * QB + jb : b * QB + jb + 1],
                                )
                                nc.tensor.matmul(
                                    pat[0 : HD + 1, :],
                                    v_aug[:, b * QB + jb, hh * 65 : hh * 65 + HD + 1],
                                    ex[:],
                                    start=(jb == 0), stop=(jb == qi),
                                )
                            patS = tpool.tile([HD + 1, 128], f32, tag="patS")
                            nc.vector.tensor_copy(patS[:], pat[0 : HD + 1, :])
                            recF = tpool.tile([1, 128], f32, tag="recF")
                            nc.vector.reciprocal(recF[:], patS[HD : HD + 1, :])
                            recS = tpool.tile([1, 128], bf16, tag="recS")
                            nc.vector.tensor_copy(recS[:], recF[:])
                            recb = ps_qk.tile([128, 128], f32, tag="qk")
                            nc.tensor.matmul(recb[0:HD, :], ones_sb[:], recS[:], start=True, stop=True)
                            nc.vector.tensor_tensor(
                                attnT[hh * HD : (hh + 1) * HD, b * S + t0 : b * S + t0 + 128],
                                patS[0:HD, :], recb[0:HD, :], mybir.AluOpType.mult,
                            )

                # ---- A2A reshard: [my 128 dims, 2048] -> [1024 dims, my 256] ----
                for s in range(NC):
                    nc.sync.dma_start(a2a_in[s], attnT[:, s * PP : (s + 1) * PP])
                nc.gpsimd.collective_compute(
                    "AllToAll", mybir.AluOpType.bypass,
                    replica_groups=[list(range(NC))],
                    ins=[a2a_in[:]], outs=[a2a_out[:]],
                )
                atf = apool.tile([128, QB, PP], bf16, tag="vT_atf")
                for s in range(NC):
                    nc.sync.dma_start(atf[:, s, :], a2a_out[s])

                # ---- Wo + LN1 (position-local, 2 chunks of 128) ----
                o_sb = apool.tile([128, 2, D], f32, tag="o_sb")
                for pc in range(2):
                    for nh in range(2):
                        ps = ps_mm.tile([128, 512], f32, tag="mm")
                        for kc in range(QB):
                            nc.tensor.matmul(
                                ps[:],
                                atf[:, kc, pc * 128 : pc * 128 + 128],
                                wo_sb[:, kc, nh * 512 : nh * 512 + 512],
                                start=(kc == 0), stop=(kc == QB - 1),
                            )
                        nc.vector.tensor_tensor(
                            o_sb[:, pc, nh * 512 : nh * 512 + 512], ps[:],
                            vbc[:, VOF["bo"] + nh * 512 : VOF["bo"] + nh * 512 + 512],
                            mybir.AluOpType.add,
                        )
                o1 = apool.tile([128, 2, D], bf16, tag="o1")
                _ln(nc, mybir, lnpool, o_sb, vbc, VOF["g1"], VOF["be1"], o1)

                # ---- FFN ----
                o1T = apool.tile([128, QB, 256], bf16, tag="o1T")
                for pc in range(2):
                    for t in range(QB):
                        ptr = ps_tr.tile([128, 128], bf16, tag="ptr")
                        nc.tensor.transpose(ptr[:], o1[:, pc, t * 128 : t * 128 + 128], id_sb[:])
                        nc.vector.tensor_copy(o1T[:, t, pc * 128 : pc * 128 + 128], ptr[:])
                f1r = apool.tile([128, 2, D // 2], bf16, tag="f1r")
                for pc in range(2):
                    ps = ps_mm.tile([128, 512], f32, tag="mm")
                    for kc in range(QB):
                        nc.tensor.matmul(
                            ps[:], o1T[:, kc, pc * 128 : pc * 128 + 128], w1_sb[:, kc, :],
                            start=(kc == 0), stop=(kc == QB - 1),
                        )
                    f1b = tpool.tile([128, 512], f32, tag="f1b")
                    nc.vector.tensor_tensor(
                        f1b[:], ps[:], vbc[:, VOF["b1"] : VOF["b1"] + 512],
                        mybir.AluOpType.add,
                    )
                    nc.scalar.activation(
                        f1r[:, pc, :], f1b[:], mybir.ActivationFunctionType.Relu
                    )
                f1rT = apool.tile([128, 4, 256], bf16, tag="o1T")
                for pc in range(2):
                    for t in range(4):
                        ptr = ps_tr.tile([128, 128], bf16, tag="ptr")
                        nc.tensor.transpose(ptr[:], f1r[:, pc, t * 128 : t * 128 + 128], id_sb[:])
                        nc.vector.tensor_copy(f1rT[:, t, pc * 128 : pc * 128 + 128], ptr[:])
                f_sb = apool.tile([128, 2, D], f32, tag="o_sb")
                for pc in range(2):
                    for nh in range(2):
                        ps = ps_mm.tile([128, 512], f32, tag="mm")
                        for kc in range(4):
                            nc.tensor.matmul(
                                ps[:], f1rT[:, kc, pc * 128 : pc * 128 + 128],
                                w2_sb[:, kc, nh * 512 : nh * 512 + 512],
                                start=(kc == 0), stop=(kc == 3),
                            )
                        nc.vector.tensor_tensor(
                            f_sb[:, pc, nh * 512 : nh * 512 + 512], ps[:],
                            vbc[:, VOF["b2"] + nh * 512 : VOF["b2"] + nh * 512 + 512],
                            mybir.AluOpType.add,
                        )
                h_own = apool.tile([128, 2, D], bf16, tag="o1")
                _ln(nc, mybir, lnpool, f_sb, vbc, VOF["g2"], VOF["be2"], h_own)

                # ---- h^T own + AllGather (skip AG on last layer) ----
                hT_own = apool.tile([128, QB, PP], bf16, tag="attnT")
                for pc in range(2):
                    for t in range(QB):
                        ptr = ps_tr.tile([128, 128], bf16, tag="ptr")
                        nc.tensor.transpose(ptr[:], h_own[:, pc, t * 128 : t * 128 + 128], id_sb[:])
                        nc.vector.tensor_copy(hT_own[:, t, pc * 128 : pc * 128 + 128], ptr[:])
                if l < nl - 1:
                    nc.sync.dma_start(
                        bass.AP(ag_in[:].tensor, ag_in[:].offset,
                                [[PP, 128], [128 * PP, QB], [1, PP]]),
                        hT_own[:],
                    )
                    nc.gpsimd.collective_compute(
                        "AllGather", mybir.AluOpType.bypass,
                        replica_groups=[list(range(NC))],
                        ins=[ag_in[:]], outs=[ag_out[:]],
                    )
                    for s in range(NC):
                        a = ag_out[s]
                        nc.sync.dma_start(
                            hT[:, :, s * PP : (s + 1) * PP],
                            bass.AP(a.tensor, a.offset, [[PP, 128], [128 * PP, QB], [1, PP]]),
                        )

            # ---- unembed ----
            wf_sb = cpool.tile([128, QB, V], bf16, tag="wf")
            a = wf_d[:]
            nc.sync.dma_start(
                wf_sb[:], bass.AP(a.tensor, a.offset, [[V, 128], [128 * V, QB], [1, V]])
            )
            for pc in range(2):
                ps = ps_mm.tile([128, V], f32, tag="mm")
                for kc in range(QB):
                    nc.tensor.matmul(
                        ps[:], hT_own[:, kc, pc * 128 : pc * 128 + 128], wf_sb[:, kc, :],
                        start=(kc == 0), stop=(kc == QB - 1),
                    )
                lo = tpool.tile([128, V], f32, tag="lo")
                nc.vector.tensor_tensor(lo[:], ps[:], bfb_sb[:], mybir.AluOpType.add)
                nc.sync.dma_start(out_d[pc * 128 : pc * 128 + 128, :], lo[:])

    _legalize_sync_waits(nc)
    return nc


def _ln(nc, mybir, pool, x_sb, vbc, g_off, b_off, out_sb):
    """LayerNorm over last dim of x_sb [128, 2, D] f32 -> out_sb [128, 2, D] bf16."""
    f32 = mybir.dt.float32
    D_ = x_sb.shape[2]
    for pc in range(2):
        x = x_sb[:, pc, :]
        mu = pool.tile([128, 1], f32, tag="ln_mu")
        nc.vector.reduce_sum(mu[:], x, axis=mybir.AxisListType.X)
        mu2 = pool.tile([128, 1], f32, tag="ln_mu2")
        nc.scalar.activation(mu2[:], mu[:], mybir.ActivationFunctionType.Copy, scale=1.0 / D_)
        xc = pool.tile([128, D_], f32, tag="ln_xc")
        nc.vector.tensor_tensor(xc[:], x, mu2[:].to_broadcast([128, D_]), mybir.AluOpType.subtract)
        sq = pool.tile([128, D_], f32, tag="ln_sq")
        nc.scalar.activation(sq[:], xc[:], mybir.ActivationFunctionType.Square)
        var = pool.tile([128, 1], f32, tag="ln_var")
        nc.vector.reduce_sum(var[:], sq[:], axis=mybir.AxisListType.X)
        eps_t = pool.tile([128, 1], f32, tag="ln_eps")
        nc.vector.memset(eps_t[:], 1e-6)
        std = pool.tile([128, 1], f32, tag="ln_std")
        nc.scalar.activation(std[:], var[:], mybir.ActivationFunctionType.Sqrt, scale=1.0 / D_, bias=eps_t[:])
        rstd = pool.tile([128, 1], f32, tag="ln_rstd")
        nc.vector.reciprocal(rstd[:], std[:])
        nc.vector.tensor_tensor(xc[:], xc[:], rstd[:].to_broadcast([128, D_]), mybir.AluOpType.mult)
        nc.vector.tensor_tensor(xc[:], xc[:], vbc[:, g_off : g_off + D_], mybir.AluOpType.mult)
        nc.vector.tensor_tensor(out_sb[:, pc, :], xc[:], vbc[:, b_off : b_off + D_], mybir.AluOpType.add)


def _graph(nl):
    if nl not in _G:
        _G[nl] = _build(nl)
    return _G[nl]


def _host_prep(ins, nl):
    import ml_dtypes

    bf = ml_dtypes.bfloat16
    f = np.float32
    x = np.asarray(ins["x"])
    pe = _pos_encoding()
    h0 = (np.asarray(ins["emb"], f)[x.reshape(-1)] * math.sqrt(D) + np.tile(pe, (B, 1)))
    hT0 = np.ascontiguousarray(h0.T.astype(bf))  # [D, 2048]

    wo = np.asarray(ins["Wo"], f)[:nl].astype(bf)
    w1 = np.asarray(ins["W1"], f)[:nl].astype(bf)
    w2 = np.asarray(ins["W2"], f)[:nl].astype(bf)
    wf = np.asarray(ins["Wf"], f).astype(bf)
    bfv = np.ascontiguousarray(np.broadcast_to(np.asarray(ins["bf"], f).reshape(1, V), (128, V)))

    # E_pad_aug^T per layer: [65, 1151]
    eT = np.zeros((nl, HD + 1, 1151), bf)
    for l in range(nl):
        El = np.asarray(ins["E"][l], f)  # [S, HD]
        eT[l, 0:HD, 0:S] = El.T.astype(bf)
        eT[l, HD, S:] = bf(NEG_E)

    vecs1 = np.zeros((nl, 6656), f)
    for l in range(nl):
        vecs1[l, 0:1024] = np.asarray(ins["bo"][l], f)
        vecs1[l, 1024:2048] = np.asarray(ins["g1"][l], f)
        vecs1[l, 2048:3072] = np.asarray(ins["be1"][l], f)
        vecs1[l, 3072:4096] = np.asarray(ins["b2"][l], f)
        vecs1[l, 4096:5120] = np.asarray(ins["g2"][l], f)
        vecs1[l, 5120:6144] = np.asarray(ins["be2"][l], f)
        vecs1[l, 6144:6656] = np.asarray(ins["b1"][l], f)
    vecs = np.ascontiguousarray(
        np.broadcast_to(vecs1[:, None, :], (nl, 128, 6656)).astype(bf)
    )
    padneg = np.zeros((128, 2 * QB), f)
    for b in range(B):
        for jb in range(QB):
            padneg[:, b * QB + jb] = np.where(
                x[b, jb * 128 : (jb + 1) * 128] == 0, PAD_BIAS, 0.0
            )
    ident = np.eye(128, dtype=bf)

    in_maps = []
    for c in range(NC):
        cols = slice(2 * c * HD, 2 * (c + 1) * HD)
        wq_c = np.ascontiguousarray(np.asarray(ins["Wq"], f)[:nl, :, cols].astype(bf))
        wk_c = np.ascontiguousarray(np.asarray(ins["Wk"], f)[:nl, :, cols].astype(bf))
        wv_c = np.ascontiguousarray(np.asarray(ins["Wv"], f)[:nl, :, cols].astype(bf))
        bq = np.zeros((nl, 128, 3), f)
        bq[:, :, 0] = np.asarray(ins["bq"], f)[:nl, cols]
        bq[:, :, 1] = np.asarray(ins["bk"], f)[:nl, cols]
        bq[:, :, 2] = np.asarray(ins["bv"], f)[:nl, cols]
        in_maps.append(
            dict(
                hT0=hT0, wq=wq_c, wk=wk_c, wv=wv_c, bqkv=bq,
                wo=wo, w1=w1, w2=w2, eT=eT, vecs=vecs,
                padneg=padneg, ident=ident, wf=wf, bfv=bfv,
            )
        )
    return in_maps


def _run_device(ins, nl=L):
    from concourse.bass_utils import run_bass_kernel_spmd

    nc = _graph(nl)
    in_maps = _host_prep(ins, nl)
    t0 = time.perf_counter()
    trace = bool(int(os.environ.get("KTRACE", "0")))
    res = run_bass_kernel_spmd(
        nc, in_maps, core_ids=list(range(NC)), trace=trace,
        **(dict(trace_cores=list(range(NC))) if trace else {}),
    )
    wall_ns = int((time.perf_counter() - t0) * 1e9)
    EXEC_NS[0] += res.exec_time_ns if res.exec_time_ns else wall_ns
    LAST["res"] = res
    logits = np.concatenate([np.asarray(r["logits"], np.float32) for r in res.results], axis=0)
    return logits.reshape(B, S, V)


def kernel(
    x, emb, Wq, bq, Wk, bk, Wv, bv, Wo, bo, W1, b1, W2, b2,
    g1, be1, g2, be2, E, Wf, bf,
):
    ins = dict(
        x=x, emb=emb, Wq=Wq, bq=bq, Wk=Wk, bk=bk, Wv=Wv, bv=bv, Wo=Wo, bo=bo,
        W1=W1, b1=b1, W2=W2, b2=b2, g1=g1, be1=be1, g2=g2, be2=be2, E=E,
        Wf=Wf, bf=bf,
    )
    return _run_device(ins, L)
